# revision 6
# baseline (speedup 1.0000x reference)
"""Multi-head attention Trainium2 kernel (8 NeuronCores, SPMD).

Problem: B=2, T=2048, d_model=768, H=12 heads, d_k=64.
  Q/K/V = x @ W.T ; scores = QK^T/sqrt(d_k); weights = softmax(scores);
  attn = weights @ V ; out = attn @ W_o.T.  Returns (out, weights).

Sharding: 8 cores = 2 batches x 4 head-groups (3 heads each).  Each core
gets pre-transposed activations x.T [768,T] and its head-group's weight
slices; it produces the softmax weights for its 3 heads stored
TRANSPOSED [3, T(k), T(q)] (so the big 50MB store is DMA-contiguous) and
a partial output [T, 768] (row-parallel W_o).  Host sums the 4 partials
per batch and transposes the weights back — host work is O(memcpy).

On-core dataflow (per head h, per q-block):
  scoresT[k,q] = KT_h(ktile).T @ QT_h          (PE, K=64)
  e = exp(0.125 * scoresT)                     (ACT, PSUM->SBUF, retained)
  attnT_aug[65,q] += Vaug(ktile).T @ e         (PE; col 64 of Vaug is ones
                                                -> row 64 = softmax sums)
  recip = 1/sums; bcast via K=1 matmul; weights = e * recip (DVE) -> DMA
  attnT = attnT_aug[0:64] * recip              (DVE)
  out[t,768] += attnT_h(ttile).T @ WoT_h       (PE, accumulated over heads)
"""

import os
import sys

for _p in ("/opt/trn_rl_repo", os.path.expanduser("~/.axon_site/_ro/trn_rl_repo")):
    if os.path.isdir(_p) and _p not in sys.path:
        sys.path.insert(0, _p)

import numpy as np
from contextlib import ExitStack

import concourse.bass as bass
import concourse.bacc as bacc
import concourse.mybir as mybir
import concourse.tile as tile

F32 = mybir.dt.float32

D = 768          # d_model
DK = 64          # head dim
HL = 3           # heads per core
DH = HL * DK     # 192 head-group width
N_CORES = 8
T_FULL = 2048
B_FULL = 2
H_FULL = 12


def _nsplits(n, max_n=512):
    """Split free dim n into chunks <= max_n (PSUM-bank aligned at 512)."""
    out = []
    off = 0
    while off < n:
        sz = min(max_n, n - off)
        out.append((off, sz))
        off += sz
    return out


def build_nc(T=T_FULL, qb=None, reps=1, do_store=True, do_scale=True, do_attn=True):
    """Build the per-core Bass program.  qb = q-block size for the softmax
    working set (exp tiles for all of k must stay resident per q-block)."""
    if qb is None:
        qb = min(1024, T)
    KTD = D // 128            # 6 contraction tiles for projections
    NKT = T // 128            # k tiles
    NQB = T // qb             # q blocks
    NTT = T // 128            # t tiles for V / out-proj

    nc = bacc.Bacc()

    xqT = nc.declare_dram_parameter("xqT", [D, T], F32, isOutput=False)
    xkT = nc.declare_dram_parameter("xkT", [D, T], F32, isOutput=False)
    xvT = nc.declare_dram_parameter("xvT", [D, T], F32, isOutput=False)
    wqT = nc.declare_dram_parameter("wqT", [D, DH], F32, isOutput=False)
    wkT = nc.declare_dram_parameter("wkT", [D, DH], F32, isOutput=False)
    wvT = nc.declare_dram_parameter("wvT", [D, DH], F32, isOutput=False)
    woT = nc.declare_dram_parameter("woT", [DH, D], F32, isOutput=False)
    wts = nc.declare_dram_parameter("wts", [HL, T, T], F32, isOutput=True)
    outp = nc.declare_dram_parameter("outp", [T, D], F32, isOutput=True)

    EXPF = mybir.ActivationFunctionType.Exp

    with TileCtx(nc) as (tc, _ctx):
      for _rep in range(reps):
       with ExitStack() as ctx:
        persist = ctx.enter_context(tc.tile_pool(name="persist", bufs=1))

        # --- constants / weights ---
        wq_sb = persist.tile([128, KTD, DH], F32, tag="wq")
        wk_sb = persist.tile([128, KTD, DH], F32, tag="wk")
        wv_sb = persist.tile([128, KTD, DH], F32, tag="wv")
        for sb, dr in ((wq_sb, wqT), (wk_sb, wkT), (wv_sb, wvT)):
            nc.sync.dma_start(out=sb, in_=dr.rearrange("(k p) n -> p k n", p=128))
        wo_sb = []
        for h in range(HL):
            t_ = persist.tile([DK, D], F32, tag=f"wo{h}", name=f"wo{h}")
            nc.sync.dma_start(out=t_, in_=woT[h * DK:(h + 1) * DK, :])
            wo_sb.append(t_)
        ones_all = persist.tile([128, 128], F32, tag="ones")
        nc.vector.memset(ones_all, 1.0)

        # --- persistent activations ---
        qt_ab = persist.tile([128, T], F32, tag="qt_ab")   # heads 0,1 (d on part)
        qt_c = persist.tile([DK, T], F32, tag="qt_c")      # head 2
        kt_ab = persist.tile([128, T], F32, tag="kt_ab")
        kt_c = persist.tile([DK, T], F32, tag="kt_c")
        # V natural [t,d] + ones column per head: [128, NKT, HL*(DK+1)]
        vaug = persist.tile([128, NKT, HL * (DK + 1)], F32, tag="vaug")
        vaug_h = vaug.rearrange("p k (h e) -> p k h e", e=DK + 1)
        nc.vector.memset(vaug_h[:, :, :, DK:DK + 1], 1.0)
        attn_sb = []
        for h in range(HL):
            attn_sb.append(persist.tile([DK, T], F32, tag=f"attn{h}", name=f"attn{h}"))

        # ---------------- Phase 1: projections ----------------
        with tc.tile_pool(name="xin", bufs=2) as xpool, \
             tc.tile_pool(name="pj", bufs=4, space="PSUM") as pj:
            for which, xdram, wsb in (("q", xqT, wq_sb), ("k", xkT, wk_sb),
                                      ("v", xvT, wv_sb)):
                x_sb = xpool.tile([128, KTD, T], F32, tag="x")
                nc.sync.dma_start(out=x_sb,
                                  in_=xdram.rearrange("(k p) n -> p k n", p=128))
                if which in ("q", "k"):
                    dst_ab, dst_c = (qt_ab, qt_c) if which == "q" else (kt_ab, kt_c)
                    # transposed layout: out[d, t] ; lhsT = W.T tile, rhs = x.T
                    for moff, msz, dst in ((0, 128, dst_ab), (128, DK, dst_c)):
                        for noff, nsz in _nsplits(T):
                            ps = pj.tile([128, 512], F32, tag="pp")
                            for kt in range(KTD):
                                nc.tensor.matmul(
                                    ps[:msz, :nsz],
                                    wsb[:, kt, moff:moff + msz],
                                    x_sb[:, kt, noff:noff + nsz],
                                    start=(kt == 0), stop=(kt == KTD - 1))
                            nc.scalar.copy(out=dst[:, noff:noff + nsz],
                                           in_=ps[:msz, :nsz])
                else:
                    # V natural layout: out[t, d] ; lhsT = x.T tile, rhs = W.T
                    for tt in range(NTT):
                        ps = pj.tile([128, DH], F32, tag="pv")
                        for kt in range(KTD):
                            nc.tensor.matmul(
                                ps,
                                x_sb[:, kt, tt * 128:(tt + 1) * 128],
                                wsb[:, kt, :],
                                start=(kt == 0), stop=(kt == KTD - 1))
                        nc.vector.tensor_copy(
                            out=vaug_h[:, tt, :, 0:DK],
                            in_=ps.rearrange("p (h e) -> p h e", e=DK))

        # ---------------- Phase 2: attention ----------------
        head_src = [(kt_ab, qt_ab, 0), (kt_ab, qt_ab, 64), (kt_c, qt_c, 0)]
        with tc.tile_pool(name="exp", bufs=NKT + 2) as epool, \
             tc.tile_pool(name="small", bufs=2) as small, \
             tc.tile_pool(name="ps_s", bufs=2, space="PSUM") as pss, \
             tc.tile_pool(name="ps_a", bufs=2, space="PSUM") as psa:
            for h in range(HL):
                K_t, Q_t, pb = head_src[h]
                for qbi in range(NQB):
                    qoff = qbi * qb
                    ps_attn = psa.tile([DK + 1, qb], F32, tag="pa")
                    exp_tiles = []
                    for kt in range(NKT):
                        ps_s = pss.tile([128, qb], F32, tag="ps")
                        for noff, nsz in _nsplits(qb):
                            nc.tensor.matmul(
                                ps_s[:, noff:noff + nsz],
                                K_t[pb:pb + DK, kt * 128:(kt + 1) * 128],
                                Q_t[pb:pb + DK, qoff + noff:qoff + noff + nsz],
                                start=True, stop=True)
                        et = epool.tile([128, qb], F32, tag="exp")
                        # weights_unnorm = exp(scores / 8)
                        nc.scalar.activation(out=et, in_=ps_s, func=EXPF,
                                             scale=0.125)
                        if do_attn:
                            for noff, nsz in _nsplits(qb):
                                nc.tensor.matmul(
                                    ps_attn[:, noff:noff + nsz],
                                    vaug_h[:, kt, h, :],
                                    et[:, noff:noff + nsz],
                                    start=(kt == 0), stop=(kt == NKT - 1),
                                    skip_group_check=True)
                        exp_tiles.append(et)
                    # row 64 of ps_attn = sums of exp over k
                    recip = small.tile([128, qb], F32, tag="recip")
                    if do_attn:
                        nc.vector.reciprocal(out=recip[DK:DK + 1, :],
                                             in_=ps_attn[DK:DK + 1, :])
                    else:
                        nc.vector.memset(recip[DK:DK + 1, :], 1.0)
                    # broadcast recip across partitions via K=1 matmul
                    ps_b = pss.tile([128, qb], F32, tag="ps")
                    for noff, nsz in _nsplits(qb):
                        nc.tensor.matmul(
                            ps_b[:, noff:noff + nsz],
                            ones_all[DK:DK + 1, :],
                            recip[DK:DK + 1, noff:noff + nsz],
                            start=True, stop=True)
                    rb = small.tile([128, qb], F32, tag="rb")
                    nc.vector.tensor_copy(out=rb, in_=ps_b)
                    # attnT (normalized) -> SBUF
                    if do_attn:
                        nc.vector.tensor_mul(attn_sb[h][:, qoff:qoff + qb],
                                             ps_attn[0:DK, :], rb[0:DK, :])
                    # normalize + store softmax weights (transposed [k, q])
                    for kt in range(NKT):
                        et = exp_tiles[kt]
                        if do_scale:
                            nc.vector.tensor_mul(et, et, rb)
                        if do_store:
                            nc.sync.dma_start(
                                out=wts[h, kt * 128:(kt + 1) * 128,
                                        qoff:qoff + qb],
                                in_=et)

        # ---------------- Phase 3: output projection ----------------
        with tc.tile_pool(name="po", bufs=2, space="PSUM") as po, \
             tc.tile_pool(name="ost", bufs=3) as ost:
            for tt in range(NTT if do_attn else 0):
                ps_o = po.tile([128, D], F32, tag="po")
                for h in range(HL):
                    for noff, nsz in _nsplits(D):
                        nc.tensor.matmul(
                            ps_o[:, noff:noff + nsz],
                            attn_sb[h][:, tt * 128:(tt + 1) * 128],
                            wo_sb[h][:, noff:noff + nsz],
                            start=(h == 0), stop=(h == HL - 1),
                            skip_group_check=True)
                ot = ost.tile([128, D], F32, tag="ot")
                nc.vector.tensor_copy(out=ot, in_=ps_o)
                nc.sync.dma_start(out=outp[tt * 128:(tt + 1) * 128, :], in_=ot)

    nc.compile()
    return nc


class TileCtx:
    """TileContext + ExitStack in one `with`."""

    def __init__(self, nc):
        self.tc = tile.TileContext(nc)
        self.ctx = ExitStack()

    def __enter__(self):
        tc = self.tc.__enter__()
        self.ctx.__enter__()
        return tc, self.ctx

    def __exit__(self, *exc):
        self.ctx.__exit__(*exc)
        return self.tc.__exit__(*exc)


def _shard_inputs(query, key, value, W_q, W_k, W_v, W_o, T=T_FULL):
    in_maps = []
    for c in range(N_CORES):
        b, g = c // 4, c % 4
        sl = slice(g * DH, (g + 1) * DH)
        in_maps.append({
            "xqT": np.ascontiguousarray(query[b].T),
            "xkT": np.ascontiguousarray(key[b].T),
            "xvT": np.ascontiguousarray(value[b].T),
            "wqT": np.ascontiguousarray(W_q[sl, :].T),
            "wkT": np.ascontiguousarray(W_k[sl, :].T),
            "wvT": np.ascontiguousarray(W_v[sl, :].T),
            "woT": np.ascontiguousarray(W_o[:, sl].T),
        })
    return in_maps


def _gather(results, T=T_FULL):
    weights = np.empty((B_FULL, H_FULL, T, T), np.float32)
    output = np.zeros((B_FULL, T, D), np.float32)
    for c in range(N_CORES):
        b, g = c // 4, c % 4
        weights[b, 3 * g:3 * g + 3] = np.asarray(results[c]["wts"]).transpose(0, 2, 1)
        output[b] += np.asarray(results[c]["outp"])
    return output, weights


def kernel(query, key, value, W_q, W_k, W_v, W_o):
    from concourse.bass_utils import run_bass_kernel_spmd

    query = np.asarray(query, np.float32)
    key = np.asarray(key, np.float32)
    value = np.asarray(value, np.float32)
    W_q = np.asarray(W_q, np.float32)
    W_k = np.asarray(W_k, np.float32)
    W_v = np.asarray(W_v, np.float32)
    W_o = np.asarray(W_o, np.float32)

    nc = build_nc(T_FULL)
    in_maps = _shard_inputs(query, key, value, W_q, W_k, W_v, W_o)
    res = run_bass_kernel_spmd(nc, in_maps, list(range(N_CORES))).results
    return _gather(res)


# revision 18
# speedup vs baseline: 5.7081x; 5.7081x over previous
"""Multi-head attention Trainium2 kernel (8 NeuronCores, SPMD).

Problem: B=2, T=2048, d_model=768, H=12 heads, d_k=64.
  Q/K/V = x @ W.T ; scores = QK^T/sqrt(d_k); weights = softmax(scores);
  attn = weights @ V ; out = attn @ W_o.T.  Returns (out, weights).

Sharding: 8 cores = 2 batches x 4 head-groups (3 heads each).  Each core
gets pre-transposed activations x.T [768,T] and its head-group's weight
slices; it produces the softmax weights for its 3 heads stored
TRANSPOSED [3, T(k), T(q)] (so the big 50MB store is DMA-contiguous) and
a partial output [T, 768] (row-parallel W_o).  Host sums the 4 partials
per batch and transposes the weights back — host work is O(memcpy).

On-core dataflow (per head h, per q-block):
  scoresT[k,q] = KT_h(ktile).T @ QT_h          (PE, K=64)
  e = exp(0.125 * scoresT)                     (ACT, PSUM->SBUF, retained)
  attnT_aug[65,q] += Vaug(ktile).T @ e         (PE; col 64 of Vaug is ones
                                                -> row 64 = softmax sums)
  recip = 1/sums; bcast via K=1 matmul; weights = e * recip (DVE) -> DMA
  attnT = attnT_aug[0:64] * recip              (DVE)
  out[t,768] += attnT_h(ttile).T @ WoT_h       (PE, accumulated over heads)
"""

import os
import sys

for _p in ("/opt/trn_rl_repo", os.path.expanduser("~/.axon_site/_ro/trn_rl_repo")):
    if os.path.isdir(_p) and _p not in sys.path:
        sys.path.insert(0, _p)

import numpy as np
from contextlib import ExitStack

import concourse.bass as bass
import concourse.bacc as bacc
import concourse.mybir as mybir
import concourse.tile as tile

F32 = mybir.dt.float32
BF16 = mybir.dt.bfloat16

D = 768          # d_model
DK = 64          # head dim
HL = 3           # heads per core
DH = HL * DK     # 192 head-group width
N_CORES = 8
T_FULL = 2048
B_FULL = 2
H_FULL = 12


def _nsplits(n, max_n=512):
    """Split free dim n into chunks <= max_n (PSUM-bank aligned at 512)."""
    out = []
    off = 0
    while off < n:
        sz = min(max_n, n - off)
        out.append((off, sz))
        off += sz
    return out


def build_nc(T=T_FULL, qb=None, reps=1, do_store=True, do_scale=True, do_attn=True, bench_mode=False, cast_store=False):
    """Build the per-core Bass program.  qb = q-block size for the softmax
    working set (exp tiles for all of k must stay resident per q-block)."""
    if qb is None:
        qb = min(1024, T)
    KTD = D // 128            # 6 contraction tiles for projections
    NKT = T // 128            # k tiles
    NQB = T // qb             # q blocks
    NTT = T // 128            # t tiles for V / out-proj

    nc = bacc.Bacc()

    xqT = nc.declare_dram_parameter("xqT", [D, T], BF16, isOutput=False)
    xkT = nc.declare_dram_parameter("xkT", [D, T], BF16, isOutput=False)
    xvT = nc.declare_dram_parameter("xvT", [D, T], BF16, isOutput=False)
    wqT = nc.declare_dram_parameter("wqT", [D, DH], BF16, isOutput=False)
    wkT = nc.declare_dram_parameter("wkT", [D, DH], BF16, isOutput=False)
    wvT = nc.declare_dram_parameter("wvT", [D, DH], BF16, isOutput=False)
    woT = nc.declare_dram_parameter("woT", [DH, D], BF16, isOutput=False)
    if bench_mode:
        wts = nc.dram_tensor("wts_int", [HL, T, T], F32).ap()
    else:
        wts = nc.declare_dram_parameter("wts", [HL, T, T], F32, isOutput=True)
    outp = nc.declare_dram_parameter("outp", [T, D], F32, isOutput=True)

    EXPF = mybir.ActivationFunctionType.Exp

    with TileCtx(nc) as (tc, _ctx):
      for _rep in range(reps):
       with ExitStack() as ctx:
        persist = ctx.enter_context(tc.tile_pool(name="persist", bufs=1))

        # --- constants / weights ---
        wq_sb = persist.tile([128, KTD, DH], BF16, tag="wq")
        wk_sb = persist.tile([128, KTD, DH], BF16, tag="wk")
        wv_sb = persist.tile([128, KTD, DH], BF16, tag="wv")
        for sb, dr in ((wq_sb, wqT), (wk_sb, wkT), (wv_sb, wvT)):
            nc.sync.dma_start(out=sb, in_=dr.rearrange("(k p) n -> p k n", p=128))
        wo_sb = []
        for h in range(HL):
            t_ = persist.tile([DK, D], BF16, tag=f"wo{h}", name=f"wo{h}")
            nc.sync.dma_start(out=t_, in_=woT[h * DK:(h + 1) * DK, :])
            wo_sb.append(t_)
        ones_all = persist.tile([128, 128], F32, tag="ones")
        nc.vector.memset(ones_all, 1.0)

        # --- persistent activations ---
        qt_ab = persist.tile([128, T], BF16, tag="qt_ab")   # heads 0,1 (d on part)
        qt_c = persist.tile([DK, T], BF16, tag="qt_c")      # head 2
        kt_ab = persist.tile([128, T], BF16, tag="kt_ab")
        kt_c = persist.tile([DK, T], BF16, tag="kt_c")
        # V natural [t,d] + ones column per head: [128, NKT, HL*(DK+1)]
        vaug = persist.tile([128, NKT, HL * (DK + 1)], BF16, tag="vaug")
        vaug_h = vaug.rearrange("p k (h e) -> p k h e", e=DK + 1)
        nc.vector.memset(vaug_h[:, :, :, DK:DK + 1], 1.0)
        attn_sb = []
        for h in range(HL):
            attn_sb.append(persist.tile([DK, T], BF16, tag=f"attn{h}", name=f"attn{h}"))

        # shared pools (opened before xin so the stack allocator gives the
        # attention pools addresses disjoint from xin -> no release deps)
        epool = ctx.enter_context(tc.tile_pool(name="exp", bufs=NKT + 8))
        stg = ctx.enter_context(tc.tile_pool(name="stg", bufs=6))
        small = ctx.enter_context(tc.tile_pool(name="small", bufs=2))
        ost = ctx.enter_context(tc.tile_pool(name="ost", bufs=3))
        psum = ctx.enter_context(tc.tile_pool(name="psum", bufs=1, space="PSUM"))

        # ---------------- Phase 1: projections ----------------
        # kt-outer accumulation: first matmul starts after the first x chunk.
        # Emission order K, Q(heads01), V lets attention(qb0,h0) start while
        # Q(head2) is deferred to the attention phase (xq stays resident).
        def proj_dt(x_sb, wsb, moff, msz, dst):
            """[d, t]-layout projection of one M-block, kt-outer accumulation."""
            chunks = _nsplits(T, 1024)
            pss_ = [psum.tile([128, csz], F32, tag="ps", bufs=2,
                              name=f"pp{j}") for j, (coff, csz) in enumerate(chunks)]
            for kt in range(KTD):
                for j, (coff, csz) in enumerate(chunks):
                    for noff, nsz in _nsplits(csz):
                        nc.tensor.matmul(
                            pss_[j][:msz, noff:noff + nsz],
                            wsb[:, kt, moff:moff + msz],
                            x_sb[:, kt, coff + noff:coff + noff + nsz],
                            start=(kt == 0), stop=(kt == KTD - 1))
            for j, (coff, csz) in enumerate(chunks):
                nc.vector.tensor_copy(out=dst[:, coff:coff + csz],
                                      in_=pss_[j][:msz, :])

        xpool = ctx.enter_context(tc.tile_pool(name="xin", bufs=2))
        x_tiles = {}
        for which, xdram in (("k", xkT), ("q", xqT), ("v", xvT)):
            t_ = xpool.tile([128, KTD, T], BF16, tag="x", name=f"x_{which}")
            x_tiles[which] = t_
        for which, xdram in (("k", xkT), ("q", xqT), ("v", xvT)):
            xdr = xdram.rearrange("(k p) n -> p k n", p=128)
            for kt in range(KTD):
                nc.sync.dma_start(out=x_tiles[which][:, kt], in_=xdr[:, kt])
        proj_dt(x_tiles["k"], wk_sb, 0, 128, kt_ab)
        proj_dt(x_tiles["k"], wk_sb, 128, DK, kt_c)
        proj_dt(x_tiles["q"], wq_sb, 0, 128, qt_ab)
        # V natural layout: out[t, d] ; emitted inside the first head's
        # kt-loop (V tile kt feeds attn tile kt) to overlap with scores.
        def proj_v(tt):
            ps = psum.tile([128, 384], F32, tag="po", bufs=2, name="pv")
            for kt in range(KTD):
                nc.tensor.matmul(
                    ps[:, :DH],
                    x_tiles["v"][:, kt, tt * 128:(tt + 1) * 128],
                    wv_sb[:, kt, :],
                    start=(kt == 0), stop=(kt == KTD - 1))
            nc.vector.tensor_copy(
                out=vaug_h[:, tt, :, 0:DK],
                in_=ps[:, :DH].rearrange("p (h e) -> p h e", e=DK))

        # ---------------- Phase 2: attention + output projection ----------------
        head_src = [(kt_ab, qt_ab, 0), (kt_ab, qt_ab, 64), (kt_c, qt_c, 0)]
        if True:
            for qbi in range(NQB):
                qoff = qbi * qb
                for h in range(HL):
                    K_t, Q_t, pb = head_src[h]
                    ps_attn = psum.tile([DK + 1, qb], F32, tag="pa", bufs=1, name="pa")
                    exp_tiles = []
                    for kt in range(NKT):
                        if qbi == 0 and h == 0:
                            proj_v(kt)
                        ps_s = psum.tile([128, qb], F32, tag="ps", bufs=2, name="ps_s")
                        for noff, nsz in _nsplits(qb):
                            nc.tensor.matmul(
                                ps_s[:, noff:noff + nsz],
                                K_t[pb:pb + DK, kt * 128:(kt + 1) * 128],
                                Q_t[pb:pb + DK, qoff + noff:qoff + noff + nsz],
                                start=True, stop=True)
                        et = epool.tile([128, qb], BF16, tag="exp")
                        # weights_unnorm = exp(scores / 8)
                        nc.scalar.activation(out=et, in_=ps_s, func=EXPF,
                                             scale=0.125)
                        if do_attn:
                            for noff, nsz in _nsplits(qb):
                                nc.tensor.matmul(
                                    ps_attn[:, noff:noff + nsz],
                                    vaug_h[:, kt, h, :],
                                    et[:, noff:noff + nsz],
                                    start=(kt == 0), stop=(kt == NKT - 1),
                                    skip_group_check=True)
                        exp_tiles.append(et)
                    # row 64 of ps_attn = sums of exp over k
                    recip = small.tile([128, qb], F32, tag="recip")
                    if do_attn:
                        nc.vector.reciprocal(out=recip[DK:DK + 1, :],
                                             in_=ps_attn[DK:DK + 1, :])
                    else:
                        nc.vector.memset(recip[DK:DK + 1, :], 1.0)
                    # broadcast recip across partitions via K=1 matmul
                    ps_b = psum.tile([128, qb], F32, tag="ps", bufs=2, name="ps_b")
                    for noff, nsz in _nsplits(qb):
                        nc.tensor.matmul(
                            ps_b[:, noff:noff + nsz],
                            ones_all[DK:DK + 1, :],
                            recip[DK:DK + 1, noff:noff + nsz],
                            start=True, stop=True)
                    rb = small.tile([128, qb], BF16, tag="rb")
                    nc.vector.tensor_copy(out=rb, in_=ps_b)
                    # attnT (normalized) -> SBUF
                    if do_attn:
                        nc.vector.tensor_mul(attn_sb[h][:, qoff:qoff + qb],
                                             ps_attn[0:DK, :], rb[0:DK, :])
                    # normalize + store softmax weights (transposed [k, q])
                    for kt in range(NKT):
                        et = exp_tiles[kt]
                        if cast_store:
                            wstage = stg.tile([128, qb], BF16, tag="wstage")
                        else:
                            wstage = stg.tile([128, qb], F32, tag="wstage")
                        if do_scale:
                            nc.vector.tensor_mul(wstage, et, rb)
                        else:
                            nc.vector.tensor_copy(out=wstage, in_=et)
                        if do_store and cast_store:
                            # bf16 -> f32 widening store (SWDGE casts)
                            nc.gpsimd.dma_start(
                                out=wts[h, kt * 128:(kt + 1) * 128,
                                        qoff:qoff + qb],
                                in_=wstage)
                        elif do_store:
                            nc.sync.dma_start(
                                out=wts[h, kt * 128:(kt + 1) * 128,
                                        qoff:qoff + qb],
                                in_=wstage)
                    if qbi == 0 and h == 0:
                        # deferred head-2 K/Q projection (overlaps h0 drain)
                        proj_dt(x_tiles["q"], wq_sb, 128, DK, qt_c)
                # output projection for this q-block's t-tiles
                for tt in range(qoff // 128, (qoff + qb) // 128):
                    if not do_attn:
                        break
                    ot = ost.tile([128, D], F32, tag="ot")
                    for noff in (0, 384):
                        ps_o = psum.tile([128, 384], F32, tag="po", bufs=2,
                                         name="ps_o")
                        for h in range(HL):
                            nc.tensor.matmul(
                                ps_o,
                                attn_sb[h][:, tt * 128:(tt + 1) * 128],
                                wo_sb[h][:, noff:noff + 384],
                                start=(h == 0), stop=(h == HL - 1),
                                skip_group_check=True)
                        nc.vector.tensor_copy(out=ot[:, noff:noff + 384],
                                              in_=ps_o)
                    nc.sync.dma_start(out=outp[tt * 128:(tt + 1) * 128, :],
                                      in_=ot)

    nc.compile()
    return nc


class TileCtx:
    """TileContext + ExitStack in one `with`."""

    def __init__(self, nc):
        self.tc = tile.TileContext(nc)
        self.ctx = ExitStack()

    def __enter__(self):
        tc = self.tc.__enter__()
        self.ctx.__enter__()
        return tc, self.ctx

    def __exit__(self, *exc):
        self.ctx.__exit__(*exc)
        return self.tc.__exit__(*exc)


def _shard_inputs(query, key, value, W_q, W_k, W_v, W_o, T=T_FULL):
    import ml_dtypes
    bf16 = ml_dtypes.bfloat16

    def cvt(x):
        return np.ascontiguousarray(x).astype(bf16)

    in_maps = []
    for c in range(N_CORES):
        b, g = c // 4, c % 4
        sl = slice(g * DH, (g + 1) * DH)
        in_maps.append({
            "xqT": cvt(query[b].T),
            "xkT": cvt(key[b].T),
            "xvT": cvt(value[b].T),
            "wqT": cvt(W_q[sl, :].T),
            "wkT": cvt(W_k[sl, :].T),
            "wvT": cvt(W_v[sl, :].T),
            "woT": cvt(W_o[:, sl].T),
        })
    return in_maps


def _gather(results, T=T_FULL):
    weights = np.empty((B_FULL, H_FULL, T, T), np.float32)
    output = np.zeros((B_FULL, T, D), np.float32)
    for c in range(N_CORES):
        b, g = c // 4, c % 4
        weights[b, 3 * g:3 * g + 3] = np.asarray(results[c]["wts"]).transpose(0, 2, 1)
        output[b] += np.asarray(results[c]["outp"])
    return output, weights


def kernel(query, key, value, W_q, W_k, W_v, W_o):
    from concourse.bass_utils import run_bass_kernel_spmd

    query = np.asarray(query, np.float32)
    key = np.asarray(key, np.float32)
    value = np.asarray(value, np.float32)
    W_q = np.asarray(W_q, np.float32)
    W_k = np.asarray(W_k, np.float32)
    W_v = np.asarray(W_v, np.float32)
    W_o = np.asarray(W_o, np.float32)

    nc = build_nc(T_FULL)
    in_maps = _shard_inputs(query, key, value, W_q, W_k, W_v, W_o)
    res = run_bass_kernel_spmd(nc, in_maps, list(range(N_CORES))).results
    return _gather(res)
